# revision 9
# baseline (speedup 1.0000x reference)
"""Trainium2 Bass kernel for nn_CrossAttention (B=4, N=M=2048, 8 heads x 64).

Sharding: 8 cores = batch(4) x sequence-half(2). Core c handles batch c//2,
query rows [ (c%2)*1024, (c%2+1)*1024 ). Context is replicated to the 2
cores of a batch pair; no cross-core communication is required.

v2 design vs the previous kernel:
  * Heads are processed in even/odd PAIRS (hp = head pair 0..3). qT/kT hold
    the even head of the pair in SBUF partitions 0:64 and the odd head in
    64:128. The two sim matmuls of a pair have K=64 and execute CONCURRENTLY
    on the PE's two 64-row groups (tile_position row tiling), halving sim
    time vs sequential K=64 matmuls.
  * The out-projection contracts 128 rows/step (head pairs) instead of 64.
  * av stationaries are padded to 128 columns so LDWEIGHTS uses FWL.
    Even-head tile: [v_e(64) | 1 | 0*63]; odd-head tile: [0*63 | 1 | v_o(64)],
    so the odd head's av lands in PSUM partitions 64:128 (den at 63) and
    normalization runs lane-aligned for both heads.
  * Softmax exp runs on BOTH the Scalar ACT engine and the Vector engine:
    a subset of tiles uses two chained custom DVE ops (deg-3 poly p~=2^y,
    then p^16 via four squarings). qT is pre-scaled by Q so that the PSUM
    sim values are already in the poly's domain; the ACT path then uses
    exp(scale*y) with scale = 1/(8*Q).
  * Per-head softmax denominators use reciprocal_approx_fast (~5x faster
    than DVE reciprocal); no ACT table switches (only tanh+exp set).
  * ctx half 2 is loaded/projected between the first head pair's two
    m-halves so its DMA+projection overlap attention compute.

Host gathers the 8 outT shards ([512, 1024] each) and transposes into the
full [4, 2048, 512] output.
"""

import math
import os
import sys

import numpy as np

sys.path.insert(0, "/opt/trn_rl_repo")

USE_DVE_EXP = os.environ.get("K_DVE_EXP", "1") == "1"
USE_FAST_RECIP = os.environ.get("K_FAST_RECIP", "1") == "1"

B, N, M = 4, 2048, 2048
DIM = 512
HEADS, DIM_HEAD = 8, 64
INNER = HEADS * DIM_HEAD
NSH = N // 2          # query rows per core
N_CORES = 8
MT = M // 128         # 16 real key tiles (+1 null)
KO = DIM // 128
HP = HEADS // 2       # head pairs

# exp decomposition: weight = exp(sim/8) = p(y)^16, y = sim*QP in psum.
# p(y) = (y*A3 + A2)*y^2 + (y + A0) ~= 2^(C*y) fitted on |y| <= 0.5005
# (c-substituted so the linear coefficient is exactly 1 -> 6 ALU ops).
_C_SUB = 1.4416762866121082
A3 = 0.16424406673365533
A2 = 0.5095200910443551
A0 = 0.99970299
QP = math.log2(math.e) / 128.0 / _C_SUB      # qT pre-scale
ACT_EXP_SCALE = 1.0 / (8.0 * QP)             # ACT path: exp(scale*y)

_COMPILED = {}
_DVE_OPS = {}
LAST_EXEC_TIME_NS = None


def _ensure_dve_ops():
    """Register the two custom DVE ops (poly, quad-square) once."""
    if _DVE_OPS:
        return _DVE_OPS
    from concourse import dve_ops
    from concourse.dve_spec import Spec, Src0, C0, C1, C2, lower, _has_src1
    from concourse.dve_uop import DveOpSpec

    def build(name, body, ref):
        if name in dve_ops._SUB_OPCODE_FOR_NAME:
            for op in dve_ops.OPS:
                if op.name == name:
                    return op
        spec = Spec(body=body, reference=ref)
        row = dve_ops._CUSTOM_DVE_ROW_BASE + len(dve_ops.OPS)
        shas = {}
        for ver in ("v3", "v4"):
            s = DveOpSpec(name=name, opcode=row, uops=lower(spec, ver=ver),
                          rd1_en=_has_src1(spec))
            shas[ver] = s.sha(ver)
        op = dve_ops.DveOp(name, spec, subdim=False, uops_sha=shas)
        dve_ops.OPS.append(op)
        dve_ops.CUSTOM_DVE_SPECS[name] = spec
        dve_ops._SUB_OPCODE_FOR_NAME[name] = row
        return op

    _y = Src0
    _poly = (_y * C0 + C1) * (_y * _y) + (_y + C2)

    def _ref_poly(in0, in1, s0, s1, imm2):
        z = in0.astype(np.float32)
        return (((z * np.float32(s0) + np.float32(s1)) * (z * z))
                + (z + np.float32(imm2))).astype(np.float32)

    _q = Src0
    for _ in range(4):
        _q = _q * _q

    def _ref_sq4(in0, in1, s0, s1, imm2):
        p = in0.astype(np.float32)
        for _ in range(4):
            p = (p * p).astype(np.float32)
        return p

    _DVE_OPS["poly"] = build("EXP2POLY_ANT", _poly, _ref_poly)
    _DVE_OPS["sq4"] = build("EXPSQ4_ANT", _q, _ref_sq4)
    return _DVE_OPS


def _build():
    import concourse.tile as tile
    from concourse import bacc, mybir

    ops = _ensure_dve_ops()

    F32 = mybir.dt.float32
    BF16 = mybir.dt.bfloat16
    Act = mybir.ActivationFunctionType

    nc = bacc.Bacc("TRN2", target_bir_lowering=False, debug=False,
                   num_devices=N_CORES)

    # x and ctx are provided pre-transposed by the host: [DIM, rows]
    x_d = nc.dram_tensor("x", [DIM, NSH], F32, kind="ExternalInput").ap()
    ctx_d = nc.dram_tensor("ctx", [DIM, M], F32, kind="ExternalInput").ap()
    wq_d = nc.dram_tensor("wq", [DIM, INNER], F32, kind="ExternalInput").ap()
    # host permutes the v-half columns parity-major: [k | v_even | v_odd]
    wkv_d = nc.dram_tensor("wkv", [DIM, 2 * INNER], F32, kind="ExternalInput").ap()
    nk_d = nc.dram_tensor("nullk", [128, 1], F32, kind="ExternalInput").ap()
    nv_d = nc.dram_tensor("nullv", [1, DIM_HEAD], F32, kind="ExternalInput").ap()
    wout_d = nc.dram_tensor("wout", [INNER, DIM], F32, kind="ExternalInput").ap()
    bout_d = nc.dram_tensor("bout", [128, 4], F32, kind="ExternalInput").ap()
    out_d = nc.dram_tensor("out", [DIM, NSH], F32, kind="ExternalOutput").ap()

    with tile.TileContext(nc) as tc:
        with (
            tc.tile_pool(name="persist", bufs=1) as P,
            tc.tile_pool(name="stage", bufs=3) as ST,
            tc.tile_pool(name="norm", bufs=2) as NB,
            tc.tile_pool(name="xstp", bufs=4) as XP,
            tc.tile_pool(name="exp", bufs=4) as EX,
            tc.tile_pool(name="gen_ps", bufs=2, space="PSUM") as PS,
            tc.tile_pool(name="acc_ps", bufs=1, space="PSUM") as PSA,
        ):
            # ---- persistent SBUF tensors ----
            xT = P.tile([128, KO, NSH], BF16, tag="xT")
            ctxT = P.tile([128, KO, M], BF16, tag="ctxT")
            wq_b = P.tile([128, KO, INNER], BF16, tag="wq")
            wkv_b = P.tile([128, KO, 2 * INNER], BF16, tag="wkv")
            wout_b = P.tile([128, HP, DIM], BF16, tag="wout")
            qT2 = P.tile([128, HP, NSH], BF16, tag="qT2")
            kT2 = P.tile([128, HP, M], BF16, tag="kT2")
            v65e = P.tile([128, MT, HP, 128], BF16, tag="v65e")
            v65o = P.tile([128, MT, HP, 128], BF16, tag="v65o")
            kT_null = P.tile([128, 128], BF16, tag="kTnull")
            v65ne = P.tile([128, 128], BF16, tag="v65ne")
            v65no = P.tile([128, 128], BF16, tag="v65no")
            avT2 = P.tile([128, HP, NSH], BF16, tag="avT2")
            outA = P.tile([128, 8, 512], BF16, tag="outA")
            bout_sb = P.tile([128, 4], F32, tag="bout")
            a0c = P.tile([128, 1], F32, tag="a0c")

            nc.vector.memset(a0c[:], A0)

            # ---- null-token constant tiles ----
            nk_st = ST.tile([128, 1], F32, tag="nk")
            nc.sync.dma_start(nk_st[:], nk_d[:])
            nc.vector.memset(kT_null[:], 0.0)
            nc.scalar.activation(kT_null[:, 0:1], nk_st[:], Act.Tanh)
            nv_st = ST.tile([1, DIM_HEAD], F32, tag="nv")
            nc.sync.dma_start(nv_st[:], nv_d[:])
            nc.vector.memset(v65ne[:], 0.0)
            nc.vector.tensor_copy(v65ne[0:1, 0:DIM_HEAD], nv_st[:])
            nc.vector.memset(v65ne[0:1, 64:65], 1.0)
            nc.vector.memset(v65no[:], 0.0)
            nc.vector.tensor_copy(v65no[0:1, 64:128], nv_st[:])
            nc.vector.memset(v65no[0:1, 0:1], 1.0)
            # ones columns of the padded v tiles
            nc.vector.memset(v65e[:, :, :, 64:128], 0.0)
            nc.vector.memset(v65e[:, :, :, 64:65], 1.0)
            nc.vector.memset(v65o[:, :, :, 0:64], 0.0)
            nc.vector.memset(v65o[:, :, :, 0:1], 1.0)

            def load_weight(w_d, w_sb, rows, c_lo, c_hi, tag, dma=nc.sync):
                for ko in range(rows // 128):
                    w_st = ST.tile([128, c_hi - c_lo], F32, tag=tag)
                    dma.dma_start(
                        w_st[:], w_d[ko * 128:(ko + 1) * 128, c_lo:c_hi])
                    nc.vector.tensor_copy(w_sb[:, ko, c_lo:c_hi], w_st[:])

            def load_T(src_ap, dstT, ch_list, dma=nc.sync):
                for ch in ch_list:
                    for ko in range(KO):
                        st = XP.tile([128, 1024], F32, tag="xst")
                        dma.dma_start(
                            st[:],
                            src_ap[ko * 128:(ko + 1) * 128,
                                   ch * 1024:(ch + 1) * 1024])
                        nc.vector.tensor_copy(
                            dstT[:, ko, ch * 1024:(ch + 1) * 1024], st[:])

            # ---- q/k projections: pair jt covers heads (2jt, 2jt+1);
            # psum partitions land exactly in the even|odd pair layout ----
            def proj_pair(w_sb, w_off, src_T, dstT, jt, cch, scale_q=False):
                ps = PS.tile([128, 1024], F32, tag="ps")
                for half in range(2):
                    for kt in range(KO):
                        nc.tensor.matmul(
                            ps[:, half * 512:(half + 1) * 512],
                            lhsT=w_sb[:, kt,
                                      w_off + jt * 128:w_off + (jt + 1) * 128],
                            rhs=src_T[:, kt,
                                      cch * 1024 + half * 512:
                                      cch * 1024 + (half + 1) * 512],
                            start=(kt == 0), stop=(kt == KO - 1))
                dst = dstT[:, jt, cch * 1024:(cch + 1) * 1024]
                nc.scalar.activation(dst, ps[:], Act.Tanh)
                if scale_q:
                    nc.vector.tensor_scalar_mul(dst, dst, QP)

            def v_proj(mts):
                for mt in mts:
                    ps = PS.tile([128, 1024], F32, tag="ps")
                    pv = ps[:, 0:512]
                    for kt in range(KO):
                        nc.tensor.matmul(
                            pv,
                            lhsT=ctxT[:, kt, mt * 128:(mt + 1) * 128],
                            rhs=wkv_b[:, kt, INNER:2 * INNER],
                            start=(kt == 0), stop=(kt == KO - 1))
                    # host-permuted v columns: [even hp-major | odd hp-major]
                    nc.vector.tensor_copy(
                        v65e[:, mt, :, 0:64],
                        pv[:, 0:256].rearrange("p (hp d) -> p hp d", d=64))
                    nc.vector.tensor_copy(
                        v65o[:, mt, :, 64:128],
                        pv[:, 256:512].rearrange("p (hp d) -> p hp d", d=64))

            # ================= front =================
            load_T(x_d, xT, [0])
            load_weight(wq_d, wq_b, DIM, 0, INNER, "wst")
            for jt in range(HP):
                proj_pair(wq_b, 0, xT, qT2, jt, 0, scale_q=True)
            load_weight(wkv_d, wkv_b, DIM, 0, INNER, "wst")
            load_weight(wkv_d, wkv_b, DIM, INNER, 2 * INNER, "wst")
            load_T(ctx_d, ctxT, [0], dma=nc.gpsimd)
            for jt in range(HP):
                proj_pair(wkv_b, 0, ctxT, kT2, jt, 0)
            v_proj(range(0, 8))
            for hp in range(HP):
                wo_st = ST.tile([128, DIM], F32, tag="wst")
                nc.scalar.dma_start(wo_st[:],
                                    wout_d[hp * 128:(hp + 1) * 128, :])
                nc.vector.tensor_copy(wout_b[:, hp, :], wo_st[:])
            nc.scalar.dma_start(bout_sb[:], bout_d[:])

            # ================= attention =================
            def attention(hp, mts, av_first, av_last):
                """Software-pipelined: sims+exps of tile t, then avs of t-1."""
                pend = []
                avt_e = attention.avt_e
                avt_o = attention.avt_o
                for mt in mts:
                    null = (mt == MT)
                    for ich in range(2):
                        on_dve = (USE_DVE_EXP and not null
                                  and ich == 1 and mt % 3 == 1)
                        ps = PS.tile([128, 1024], F32, tag="ps")
                        if null:
                            lhs_e = kT_null[0:64, :]
                            lhs_o = kT_null[64:128, :]
                        else:
                            lhs_e = kT2[0:64, hp, mt * 128:(mt + 1) * 128]
                            lhs_o = kT2[64:128, hp, mt * 128:(mt + 1) * 128]
                        nc.tensor.matmul(
                            ps[:, 0:512], lhsT=lhs_e,
                            rhs=qT2[0:64, hp, ich * 512:(ich + 1) * 512],
                            start=True, stop=True)
                        nc.tensor.matmul(
                            ps[:, 512:1024], lhsT=lhs_o,
                            rhs=qT2[64:128, hp, ich * 512:(ich + 1) * 512],
                            start=True, stop=True)
                        expT = EX.tile([128, 1024], BF16, tag="expT")
                        if on_dve:
                            nc.vector._custom_dve(
                                ops["poly"], out=ps[:], in0=ps[:],
                                s0=A3, s1=A2, imm2=A0)
                            nc.vector._custom_dve(
                                ops["sq4"], out=expT[:], in0=ps[:])
                        else:
                            nc.scalar.activation(expT[:], ps[:], Act.Exp,
                                                 scale=ACT_EXP_SCALE)
                        pend.append((mt, ich, expT))
                        if len(pend) > 2:
                            emit_av(hp, avt_e, avt_o, pend.pop(0),
                                    av_first, av_last)
                for item in pend:
                    emit_av(hp, avt_e, avt_o, item, av_first, av_last)

            def emit_av(hp, avt_e, avt_o, item, av_first, av_last):
                mt, ich, expT = item
                null = (mt == MT)
                lv_e = v65ne[:] if null else v65e[:, mt, hp, :]
                lv_o = v65no[:] if null else v65o[:, mt, hp, :]
                nc.tensor.matmul(
                    avt_e[:, ich * 512:(ich + 1) * 512], lhsT=lv_e,
                    rhs=expT[:, 0:512],
                    start=(mt == av_first), stop=(mt == av_last))
                nc.tensor.matmul(
                    avt_o[:, ich * 512:(ich + 1) * 512], lhsT=lv_o,
                    rhs=expT[:, 512:1024],
                    start=(mt == av_first), stop=(mt == av_last))

            def norm(hp):
                """avT2[:, hp, :] = avt/den (even: rows 0:64 den@64;
                odd: rows 64:128, den@0)."""
                avt_e, avt_o = attention.avt_e, attention.avt_o
                for par, avt, dr, rows in (
                        (0, avt_e, 64, slice(0, 64)),
                        (1, avt_o, 0, slice(64, 128))):
                    den = NB.tile([128, NSH], F32, tag="den")
                    if USE_FAST_RECIP:
                        nc.vector.reciprocal_approx_fast(
                            den[dr:dr + 1, :], avt[dr:dr + 1, :])
                    else:
                        nc.vector.reciprocal(
                            den[dr:dr + 1, :], avt[dr:dr + 1, :])
                    if dr != 0:
                        den0 = NB.tile([1, NSH], F32, tag="den0")
                        nc.sync.dma_start(den0[0:1, :], den[dr:dr + 1, :])
                        src0 = den0[0:1, :]
                    else:
                        src0 = den[0:1, :]
                    denb = NB.tile([128, NSH], F32, tag="denb")
                    nc.gpsimd.partition_broadcast(denb[:], src0)
                    nc.vector.tensor_mul(
                        avT2[rows, hp, :], avt[rows, :], denb[rows, :])

            # hp0 first half, then ctx half 2 + its projections (overlap),
            # then the rest.
            attention.avt_e = PSA.tile([128, NSH], F32, tag="avte")
            attention.avt_o = PSA.tile([128, NSH], F32, tag="avto")
            attention(0, range(0, 8), 0, MT)
            load_T(ctx_d, ctxT, [1], dma=nc.gpsimd)
            for jt in range(HP):
                proj_pair(wkv_b, 0, ctxT, kT2, jt, 1)
            v_proj(range(8, MT))
            attention(0, range(8, MT + 1), 0, MT)
            norm(0)
            for hp in range(1, HP):
                attention.avt_e = PSA.tile([128, NSH], F32, tag="avte")
                attention.avt_o = PSA.tile([128, NSH], F32, tag="avto")
                attention(hp, range(MT + 1), 0, MT)
                if hp < HP - 1:
                    norm(hp)

            # out-projection partials for hp 0..2 (run during hp3 norm)
            outT_d = out_d.rearrange("(co p) i -> p co i", p=128)
            for r in range(8):
                ct, ich = r // 2, r % 2
                ps_o = PS.tile([128, 1024], F32, tag="ps")
                pso = ps_o[:, 0:512]
                for hp in range(HP - 1):
                    nc.tensor.matmul(
                        pso,
                        lhsT=wout_b[:, hp, ct * 128:(ct + 1) * 128],
                        rhs=avT2[:, hp, ich * 512:(ich + 1) * 512],
                        start=(hp == 0), stop=(hp == HP - 2))
                nc.vector.tensor_add(
                    outA[:, r, :], pso,
                    bout_sb[:, ct:ct + 1].to_broadcast((128, 512)))
            norm(HP - 1)
            for r in range(8):
                ct, ich = r // 2, r % 2
                ps_b = PS.tile([128, 1024], F32, tag="ps")
                psb = ps_b[:, 0:512]
                nc.tensor.matmul(
                    psb,
                    lhsT=wout_b[:, HP - 1, ct * 128:(ct + 1) * 128],
                    rhs=avT2[:, HP - 1, ich * 512:(ich + 1) * 512],
                    start=True, stop=True)
                ost = ST.tile([128, 512], F32, tag="ost")
                nc.vector.tensor_add(ost[:], psb, outA[:, r, :])
                deng = nc.sync if r % 2 == 0 else nc.scalar
                deng.dma_start(
                    outT_d[:, ct, ich * 512:(ich + 1) * 512], ost[:])

    nc.compile()
    return nc


def _get_compiled():
    if "nc" not in _COMPILED:
        _COMPILED["nc"] = _build()
    return _COMPILED["nc"]


def kernel(x, context, Wq, Wkv, null_k, null_v, Wout, bout):
    global LAST_EXEC_TIME_NS
    from concourse.bass_utils import run_bass_kernel_spmd

    x = np.ascontiguousarray(np.asarray(x, dtype=np.float32))
    context = np.ascontiguousarray(np.asarray(context, dtype=np.float32))
    nk = np.tile(np.asarray(null_k, np.float32).reshape(64, 1), (2, 1)).copy()
    nv = np.asarray(null_v, np.float32).reshape(1, 64)
    bout_r = np.asarray(bout, np.float32).reshape(4, 128).T.copy()
    wq = np.ascontiguousarray(np.asarray(Wq, np.float32))
    wkv = np.asarray(Wkv, np.float32)
    # permute v columns parity-major: [v_even hp-major | v_odd hp-major]
    wv = wkv[:, INNER:].reshape(DIM, HEADS, DIM_HEAD)
    wkv_p = np.concatenate(
        [wkv[:, :INNER],
         wv[:, 0::2].reshape(DIM, INNER // 2),
         wv[:, 1::2].reshape(DIM, INNER // 2)], axis=1)
    wkv_p = np.ascontiguousarray(wkv_p)
    wout = np.ascontiguousarray(np.asarray(Wout, np.float32))

    in_maps = []
    ctxT_all = [np.ascontiguousarray(context[b].T) for b in range(B)]
    for c in range(N_CORES):
        b, j = c // 2, c % 2
        in_maps.append({
            "x": np.ascontiguousarray(x[b, j * NSH:(j + 1) * NSH, :].T),
            "ctx": ctxT_all[b],
            "wq": wq,
            "wkv": wkv_p,
            "nullk": nk,
            "nullv": nv,
            "wout": wout,
            "bout": bout_r,
        })

    nc = _get_compiled()
    res = run_bass_kernel_spmd(nc, in_maps, core_ids=list(range(N_CORES)))
    LAST_EXEC_TIME_NS = res.exec_time_ns

    out = np.empty((B, N, DIM), np.float32)
    for c in range(N_CORES):
        b, j = c // 2, c % 2
        out[b, j * NSH:(j + 1) * NSH, :] = res.results[c]["out"].T
    return out
